# revision 1
# baseline (speedup 1.0000x reference)
"""Multi-head attention (B=4, S=2048, E=768, H=12) on 8 trn2 NeuronCores.

Sharding: tensor-parallel over heads x data-parallel over batch. Core c
handles batch b=c//2 and heads 6*(c%2)..6*(c%2)+5 (all 2048 queries). Each
core emits a partial output projection (its 6 heads' contribution); the two
cores of a batch pair are summed on the host during unsharding. The bias is
added on device by the even core only (odd cores receive a zero bias).

Layouts: matmul operands keep "feature on partitions" so that
  - the qkv projection emits Q^T/K^T directly (lhsT=w^T chunk, rhs=x^T chunk),
  - Q@K^T emits S^T = [k, q] (lhsT=K^T slice, rhs=Q^T slice, contract d=64),
  - softmax row sums come from a ones-column appended to V (AV matmul M=65),
  - attention output lands as outT [e', q] - exactly the lhsT the output
    projection wants.
exp() runs on ScalarE over [128, 2048] PSUM strips with the 1/sqrt(d) scale
folded into the activation's affine input; max-subtraction is skipped
(logits are ~N(0,1), exp cannot overflow).

Dtypes: all matmul operands are fp16 (1 cyc/row like bf16 but 4x the
mantissa; every intermediate here is well inside fp16 range) with fp32 PSUM
accumulation. Softmax normalization divides via a reciprocal that is
reshaped [1,512]->[128,4] through a DRAM bounce (DVE reciprocal cost is
free-size x lanes), and the same DRAM bounce provides the partition
broadcast of 1/sum.

Two environment workarounds (this walrus build): sync-waits are split one
per instruction onto NoOps (_split_waits, _TC), and a 16-matmul warm-up
burst is pinned - via its PSUM tile's WAR dependency on stage A's banks -
to land right after the stage A->B stall, so the PE HAM clock gate stays at
2.4 GHz through the attention phase (a single >2us PE gap otherwise locks
the clock at 1.2 GHz for the rest of the kernel).
"""

import numpy as np

import concourse.bass as bass
import concourse.tile as tile
from concourse import mybir
from concourse.bass_utils import run_bass_kernel_spmd
from concourse.masks import make_identity
from concourse.tile import ScopedClock

B, S, E, H, D = 4, 2048, 768, 12, 64
NCORES = 8
HL = 6               # heads per core
FL = HL * D          # 384 local feature dim
SCALE = D ** -0.5
FP = mybir.dt.float32
FR = mybir.dt.float32r
F16 = mybir.dt.float16
BF = mybir.dt.bfloat16
P = 128

ET = E // P          # 6 e-chunks of 128
FT = FL // P         # 3 local f-tiles of 128
NKT = S // P         # 16 k-tiles of 128
NQC = S // 512       # 4 q-chunks of 512
NST = S // P         # 16 s-tiles
DV = D + 1           # 65: V plus ones column


class _TC(tile.TileContext):
    """TileContext with the end-of-kernel drain's sem waits split one per
    instruction (this walrus build's CTRL_NO_STRUCT encoding holds only one
    sync wait; the stock drain carries one wait per outstanding proc)."""

    def _drain_and_barrier(self, tick_clock, wait_clock):
        probe = self.nc.sync.nop()
        wait_clock.add_sem_waits(
            probe.ins, ScopedClock({None: tick_clock.global_clock})
        )
        si = probe.ins.sync_info
        waits = list(si.on_wait) if si is not None else []
        if len(waits) > 1:
            si.on_wait = waits[:1]
            for w in waits[1:]:
                n = self.nc.sync.nop()
                n.ins.sync_info = type(si)(on_wait=[w], on_update=[])
        self.nc.sync.drain()
        self.nc.all_engine_barrier()
        popped = self.nc._tile_sem_poison_stack.pop()
        assert popped is self._sem_poison
        self.nc.clear_and_free_semaphores(list(self.sems.allocated().values()))
        self.nc.all_engine_barrier()


def _split_waits(nc):
    """This walrus build accepts at most one sync-wait per TPB instruction
    (two on EventSemaphore). Tile emits up to 2-3. Hoist the extras onto
    same-engine NoOps inserted immediately before the instruction."""
    ctr = [0]
    for f in nc.m.functions:
        for bb in f.blocks:
            out = []
            changed = False
            for inst in bb.instructions:
                si = getattr(inst, "sync_info", None)
                if si is not None and si.on_wait:
                    cap = 2 if isinstance(inst, mybir.InstEventSemaphore) else 1
                    waits = list(si.on_wait)
                    if len(waits) > cap:
                        changed = True
                        for w in waits[:-cap]:
                            ctr[0] += 1
                            out.append(
                                mybir.InstNoOp(
                                    name=f"WSPLIT-{ctr[0]}",
                                    engine=inst.engine,
                                    ins=[],
                                    outs=[],
                                    sync_info=mybir.SyncInfo(
                                        on_wait=[w], on_update=[]
                                    ),
                                    bass_nofuse=True,
                                )
                            )
                        si.on_wait = waits[-cap:]
                        inst.sync_info = si
                out.append(inst)
            if changed:
                bb.instructions = out


def build(n_reps=1):
    nc = bass.Bass()
    xb = nc.dram_tensor("xb", [S, E], FP, kind="ExternalInput")
    wqkvT = nc.dram_tensor("wqkvT", [E, 3 * FL], F16, kind="ExternalInput")
    wprojT = nc.dram_tensor("wprojT", [FL, E], F16, kind="ExternalInput")
    biasb = nc.dram_tensor("biasb", [P, E], FP, kind="ExternalInput")
    identd = nc.dram_tensor("identd", [P, P], FP, kind="ExternalInput")
    out = nc.dram_tensor("out", [S, E], FP, kind="ExternalOutput")

    Exp = mybir.ActivationFunctionType.Exp

    from contextlib import ExitStack

    with _TC(nc) as tc, ExitStack() as stack:
        consts = stack.enter_context(tc.tile_pool(name="consts", bufs=1))
        persist = stack.enter_context(tc.tile_pool(name="persist", bufs=1))

        ident = consts.tile([P, P], FP)
        nc.sync.dma_start(ident[:], identd[:])
        bias_sb = consts.tile([P, E], FP)

        wproj_sb = [
            consts.tile([P, E], F16, tag=f"wproj{c}", name=f"wproj{c}")
            for c in range(FT)
        ]


        # persistent activations
        qT = [persist.tile([P, S], F16, tag=f"qT{t}", name=f"qT{t}") for t in range(FT)]
        kT = [persist.tile([P, S], F16, tag=f"kT{t}", name=f"kT{t}") for t in range(FT)]
        vp = [persist.tile([P, HL * DV], F16, tag=f"vp{t}", name=f"vp{t}") for t in range(NST)]
        outT = [persist.tile([P, S], F16, tag=f"outT{t}", name=f"outT{t}") for t in range(FT)]

        for _rep in range(n_reps):
            # ---------------- Stage A: transposes + projections ----------------
            with tc.tile_pool(name="stagea", bufs=1) as stagea, \
                 tc.tile_pool(name="xload", bufs=4) as xload, \
                 tc.tile_pool(name="tr_psum", bufs=4, space="PSUM") as tr_psum, \
                 tc.tile_pool(name="mm_psum", bufs=3, space="PSUM") as mm_psum:

                wqkv_sb = [
                    stagea.tile([P, 3 * FL], F16, tag=f"wqkv{c}", name=f"wqkv{c}")
                    for c in range(ET)
                ]
                xbT = [
                    stagea.tile([P, S], F16, tag=f"xbT{c}", name=f"xbT{c}")
                    for c in range(ET)
                ]

                # x^T tiles via PE transpose (fp32)
                for t in range(NST):
                    xt = xload.tile([P, E], FP, tag="xt")
                    nc.sync.dma_start(xt[:], xb[P * t : P * (t + 1), :])
                    if t == 1:
                        for c in range(ET):
                            nc.sync.dma_start(
                                wqkv_sb[c][:], wqkvT[P * c : P * (c + 1), :]
                            )
                        for c in range(FT):
                            nc.sync.dma_start(
                                wproj_sb[c][:], wprojT[P * c : P * (c + 1), :]
                            )
                        nc.sync.dma_start(bias_sb[:], biasb[:])
                    for c in range(ET):
                        pt = tr_psum.tile([P, P], FP, tag="tr")
                        nc.tensor.transpose(pt[:], xt[:, P * c : P * (c + 1)], ident[:])
                        nc.vector.tensor_copy(xbT[c][:, P * t : P * (t + 1)], pt[:])

                # Q^T [384, S] then K^T [384, S]
                for which, dst in ((0, qT), (1, kT)):
                    for ft in range(FT):
                        for j in range(NQC):
                            pq = mm_psum.tile([P, 512], FP, tag="mm")
                            for c in range(ET):
                                nc.tensor.matmul(
                                    pq[:],
                                    (wqkv_sb[c][:, FL * which + P * ft : FL * which + P * (ft + 1)]),
                                    (xbT[c][:, 512 * j : 512 * (j + 1)]),
                                    start=(c == 0),
                                    stop=(c == ET - 1),
                                )
                            nc.vector.tensor_copy(dst[ft][:, 512 * j : 512 * (j + 1)], pq[:])

                # V [S, 384] natural layout + interleaved ones columns
                for t in range(NST):
                    pv = mm_psum.tile([P, 512], FP, tag="mm")
                    for c in range(ET):
                        nc.tensor.matmul(
                            pv[:, :FL],
                            (xbT[c][:, P * t : P * (t + 1)]),
                            (wqkv_sb[c][:, 2 * FL : 3 * FL]),
                            start=(c == 0),
                            stop=(c == ET - 1),
                        )
                    v3 = vp[t].rearrange("p (h d) -> p h d", d=DV)
                    nc.vector.tensor_copy(
                        v3[:, :, 0:D], pv[:, :FL].rearrange("p (h d) -> p h d", d=D)
                    )
                    nc.vector.memset(v3[:, :, D : D + 1], 1.0)

            # ---------------- Stage B: attention, half-sequence strips ------
            HQ = S // 2  # 1024 queries per strip
            with tc.tile_pool(name="s_psum", bufs=2, space="PSUM") as s_psum, \
                 tc.tile_pool(name="o_psum", bufs=4, space="PSUM") as o_psum, \
                 tc.tile_pool(name="expst", bufs=8) as expst, \
                 tc.tile_pool(name="smalls", bufs=6) as smalls, \
                 tc.tile_pool(name="invdram", bufs=6, space="DRAM") as invdram:

                # HAM re-warm burst: a continuous PE stretch pinned (via the
                # o-pool's WAR on stage A's psum banks) to land right after
                # the A->B evacuation stall, so stage B runs at full clock.
                warm = o_psum.tile([P, 512], FP, tag="o", name="warm")
                for w in range(16):
                    nc.tensor.matmul(
                        warm[:],
                        kT[0][0:D, 0:P],
                        qT[0][0:D, 0:512],
                        start=(w == 0),
                        stop=(w == 15),
                    )
                for h in range(HL):
                    hb = (D * h) % P
                    ht = (D * h) // P
                    for half in range(2):
                        q0 = HQ * half
                        po = [
                            o_psum.tile([P, 512], FP, tag="o", name=f"po{h}_{half}_{j}")
                            for j in range(2)
                        ]
                        for i in range(NKT):
                            ps = s_psum.tile([P, HQ], FP, tag="s")
                            for j in range(2):
                                nc.tensor.matmul(
                                    ps[:, 512 * j : 512 * (j + 1)],
                                    (kT[ht][hb : hb + D, P * i : P * (i + 1)]),
                                    (qT[ht][hb : hb + D, q0 + 512 * j : q0 + 512 * (j + 1)]),
                                    start=True,
                                    stop=True,
                                )
                            es = expst.tile([P, HQ], F16, tag="e")
                            nc.scalar.activation(es[:], ps[:], Exp, scale=SCALE)
                            for j in range(2):
                                nc.tensor.matmul(
                                    po[j][:DV, :],
                                    vp[i][:, DV * h : DV * (h + 1)],
                                    es[:, 512 * j : 512 * (j + 1)],
                                    start=(i == 0),
                                    stop=(i == NKT - 1),
                                )
                        for j in range(2):
                            # sums row -> DRAM, reread as [128,4] so the
                            # reciprocal runs 128-wide (free-size cost), then
                            # bounce back through DRAM for the partition
                            # broadcast (SBUF sources can't have step-0
                            # partitions; DRAM can).
                            srow = smalls.tile([1, 512], FP, tag="srow")
                            nc.vector.tensor_copy(srow[:], po[j][D : D + 1, :])
                            sd = invdram.tile([1, 512], FP, tag="sd")
                            nc.sync.dma_start(sd[:], srow[:])
                            s4 = smalls.tile([P, 4], FP, tag="s4")
                            nc.sync.dma_start(
                                s4[:], sd.rearrange("a (p f) -> (a p) f", p=P)
                            )
                            inv4 = smalls.tile([P, 4], FP, tag="inv4")
                            nc.vector.reciprocal(inv4[:], s4[:])
                            invd = invdram.tile([1, 512], FP, tag="invd")
                            nc.sync.dma_start(
                                invd.rearrange("a (p f) -> (a p) f", p=P), inv4[:]
                            )
                            inv64 = smalls.tile([D, 512], FP, tag="inv64")
                            nc.sync.dma_start(
                                inv64[:], invd[0:1, :].to_broadcast((D, 512))
                            )
                            nc.vector.tensor_mul(
                                outT[ht][hb : hb + D, q0 + 512 * j : q0 + 512 * (j + 1)],
                                po[j][0:D, :],
                                inv64[:],
                            )

            # ---------------- Stage C: partial output projection + bias --------
            with tc.tile_pool(name="f_psum", bufs=2, space="PSUM") as f_psum, \
                 tc.tile_pool(name="osb", bufs=4) as osb:
                for t in range(NST):
                    pf1 = f_psum.tile([P, 512], FP, tag="f1")
                    pf2 = f_psum.tile([P, 512], FP, tag="f2")
                    for c in range(FT):
                        nc.tensor.matmul(
                            pf1[:],
                            (outT[c][:, P * t : P * (t + 1)]),
                            (wproj_sb[c][:, 0:512]),
                            start=(c == 0),
                            stop=(c == FT - 1),
                        )
                    for c in range(FT):
                        nc.tensor.matmul(
                            pf2[:, :256],
                            (outT[c][:, P * t : P * (t + 1)]),
                            (wproj_sb[c][:, 512:E]),
                            start=(c == 0),
                            stop=(c == FT - 1),
                        )
                    ot = osb.tile([P, E], FP, tag="ot")
                    nc.vector.tensor_add(ot[:, 0:512], pf1[:], bias_sb[:, 0:512])
                    nc.vector.tensor_add(ot[:, 512:E], pf2[:, :256], bias_sb[:, 512:E])
                    nc.sync.dma_start(out[P * t : P * (t + 1), :], ot[:])

    _split_waits(nc)
    return nc


_CACHE = {}


def _get_nc():
    if "nc" not in _CACHE:
        _CACHE["nc"] = build()
    return _CACHE["nc"]


def make_in_maps(x, w_qkv, w_proj, b_proj):
    x = np.asarray(x, dtype=np.float32)
    w_qkv = np.asarray(w_qkv, np.float32)
    w_proj = np.asarray(w_proj, np.float32)
    b_proj = np.asarray(b_proj, np.float32)
    bias0 = np.ascontiguousarray(np.broadcast_to(b_proj, (P, E)))
    biasz = np.zeros((P, E), np.float32)
    in_maps = []
    for c in range(NCORES):
        b, half = c // 2, c % 2
        heads = range(HL * half, HL * half + HL)
        rows = (
            [E * 0 + D * h + d for h in heads for d in range(D)]
            + [E * 1 + D * h + d for h in heads for d in range(D)]
            + [E * 2 + D * h + d for h in heads for d in range(D)]
        )
        wqkvT_l = np.ascontiguousarray(w_qkv[rows, :].T).astype(np.float16)
        wprojT_l = np.ascontiguousarray(w_proj[:, rows[: FL]].T).astype(np.float16)
        in_maps.append(
            {
                "identd": np.eye(P, dtype=np.float32),
                "xb": np.ascontiguousarray(x[b]),
                "wqkvT": wqkvT_l,
                "wprojT": wprojT_l,
                "biasb": bias0 if half == 0 else biasz,
            }
        )
    return in_maps


def assemble(results):
    outp = np.empty((B, S, E), np.float32)
    for b in range(B):
        outp[b] = results[2 * b]["out"] + results[2 * b + 1]["out"]
    return outp


def kernel(x, w_qkv, w_proj, b_proj):
    nc = _get_nc()
    in_maps = make_in_maps(x, w_qkv, w_proj, b_proj)
    res = run_bass_kernel_spmd(nc, in_maps, core_ids=list(range(NCORES)))
    return assemble(res.results)



# revision 2
# speedup vs baseline: 1.0059x; 1.0059x over previous
"""Multi-head attention (B=4, S=2048, E=768, H=12) on 8 trn2 NeuronCores.

Sharding: tensor-parallel over heads x data-parallel over batch. Core c
handles batch b=c//2 and heads 6*(c%2)..6*(c%2)+5 (all 2048 queries). Each
core emits a partial output projection (its 6 heads' contribution); the two
cores of a batch pair are summed on the host during unsharding. The bias is
added on device by the even core only (odd cores receive a zero bias).

Layouts: matmul operands keep "feature on partitions" so that
  - the qkv projection emits Q^T/K^T directly (lhsT=w^T chunk, rhs=x^T chunk),
  - Q@K^T emits S^T = [k, q] (lhsT=K^T slice, rhs=Q^T slice, contract d=64),
  - softmax row sums come from a ones-column appended to V (AV matmul M=65),
  - attention output lands as outT [e', q] - exactly the lhsT the output
    projection wants.

Schedule (v2, PE/Scalar co-saturation): the kernel is jointly limited by the
PE (matmul stream, ~565k cycles) and the Scalar engine (exp over 25.2M
logits, ~212us). Stage A pipelines x-loads, PE transposes and Q^T/K^T
chunks so the PE never waits on DMA (weights ride a second DMA queue on the
scalar engine). The V projection is woven into the first attention
head-half's groups (one V k-tile per group, PSUM: s_psum 4 banks + v_psum 2
+ first-head o_pool 2 = 8) so the exp stream starts ~50us earlier than a
separate V phase would allow. exp() runs on ScalarE over [128, 1024] PSUM
strips with the 1/sqrt(d) scale folded into the activation's affine input;
max-subtraction is skipped (logits are ~N(0,1), exp cannot overflow).

Dtypes: all matmul operands are fp16 (1 cyc/row like bf16 but 4x the
mantissa; every intermediate here is well inside fp16 range) with fp32 PSUM
accumulation. Softmax normalization divides via a reciprocal that is
reshaped [1,512]->[128,4] through a DRAM bounce (DVE reciprocal cost is
free-size x lanes), and the same DRAM bounce provides the partition
broadcast of 1/sum.

Environment workarounds (this walrus build): sync-waits are split one per
instruction onto NoOps (_split_waits, _TC).
"""

import numpy as np

from contextlib import ExitStack

import concourse.bass as bass
import concourse.tile as tile
from concourse import mybir
from concourse.bass_utils import run_bass_kernel_spmd
from concourse.tile import ScopedClock

B, S, E, H, D = 4, 2048, 768, 12, 64
NCORES = 8
HL = 6               # heads per core
FL = HL * D          # 384 local feature dim
SCALE = D ** -0.5
FP = mybir.dt.float32
F16 = mybir.dt.float16
P = 128

ET = E // P          # 6 e-chunks of 128
FT = FL // P         # 3 local f-tiles of 128
NKT = S // P         # 16 k-tiles of 128
NQC = S // 512       # 4 q-chunks of 512
NST = S // P         # 16 s-tiles
DV = D + 1           # 65: V plus ones column
HQ = S // 2          # 1024 queries per strip


class _TC(tile.TileContext):
    """TileContext with the end-of-kernel drain's sem waits split one per
    instruction (this walrus build's CTRL_NO_STRUCT encoding holds only one
    sync wait; the stock drain carries one wait per outstanding proc)."""

    def _drain_and_barrier(self, tick_clock, wait_clock):
        probe = self.nc.sync.nop()
        wait_clock.add_sem_waits(
            probe.ins, ScopedClock({None: tick_clock.global_clock})
        )
        si = probe.ins.sync_info
        waits = list(si.on_wait) if si is not None else []
        if len(waits) > 1:
            si.on_wait = waits[:1]
            for w in waits[1:]:
                n = self.nc.sync.nop()
                n.ins.sync_info = type(si)(on_wait=[w], on_update=[])
        self.nc.sync.drain()
        self.nc.all_engine_barrier()
        popped = self.nc._tile_sem_poison_stack.pop()
        assert popped is self._sem_poison
        self.nc.clear_and_free_semaphores(list(self.sems.allocated().values()))
        self.nc.all_engine_barrier()


def _split_waits(nc):
    """This walrus build accepts at most one sync-wait per TPB instruction
    (two on EventSemaphore). Tile emits up to 2-3. Hoist the extras onto
    same-engine NoOps inserted immediately before the instruction."""
    ctr = [0]
    for f in nc.m.functions:
        for bb in f.blocks:
            out = []
            changed = False
            for inst in bb.instructions:
                si = getattr(inst, "sync_info", None)
                if si is not None and si.on_wait:
                    cap = 2 if isinstance(inst, mybir.InstEventSemaphore) else 1
                    waits = list(si.on_wait)
                    if len(waits) > cap:
                        changed = True
                        for w in waits[:-cap]:
                            ctr[0] += 1
                            out.append(
                                mybir.InstNoOp(
                                    name=f"WSPLIT-{ctr[0]}",
                                    engine=inst.engine,
                                    ins=[],
                                    outs=[],
                                    sync_info=mybir.SyncInfo(
                                        on_wait=[w], on_update=[]
                                    ),
                                    bass_nofuse=True,
                                )
                            )
                        si.on_wait = waits[-cap:]
                        inst.sync_info = si
                out.append(inst)
            if changed:
                bb.instructions = out


def build():
    nc = bass.Bass()
    xb = nc.dram_tensor("xb", [S, E], FP, kind="ExternalInput")
    wqkvT = nc.dram_tensor("wqkvT", [E, 3 * FL], F16, kind="ExternalInput")
    wprojT = nc.dram_tensor("wprojT", [FL, E], F16, kind="ExternalInput")
    biasb = nc.dram_tensor("biasb", [P, E], FP, kind="ExternalInput")
    identd = nc.dram_tensor("identd", [P, P], FP, kind="ExternalInput")
    out = nc.dram_tensor("out", [S, E], FP, kind="ExternalOutput")

    Exp = mybir.ActivationFunctionType.Exp

    with _TC(nc) as tc, ExitStack() as stack:
        consts = stack.enter_context(tc.tile_pool(name="consts", bufs=1))
        persist = stack.enter_context(tc.tile_pool(name="persist", bufs=1))

        ident = consts.tile([P, P], FP)
        nc.sync.dma_start(ident[:], identd[:])
        bias_sb = consts.tile([P, E], FP)

        wqkv_sb = [
            consts.tile([P, 3 * FL], F16, tag=f"wqkv{c}", name=f"wqkv{c}")
            for c in range(ET)
        ]
        wproj_sb = [
            consts.tile([P, E], F16, tag=f"wproj{c}", name=f"wproj{c}")
            for c in range(FT)
        ]
        # weights ride the scalar engine's DMA queue so they never delay
        # the x tiles on the sync queue (ScalarE is idle through stage A).
        for c in range(ET):
            nc.scalar.dma_start(wqkv_sb[c][:], wqkvT[P * c : P * (c + 1), :])
        for c in range(FT):
            nc.scalar.dma_start(wproj_sb[c][:], wprojT[P * c : P * (c + 1), :])
        nc.scalar.dma_start(bias_sb[:], biasb[:])

        # persistent activations
        xbT = [persist.tile([P, S], F16, tag=f"xbT{c}", name=f"xbT{c}") for c in range(ET)]
        qT = [persist.tile([P, S], F16, tag=f"qT{t}", name=f"qT{t}") for t in range(FT)]
        kT = [persist.tile([P, S], F16, tag=f"kT{t}", name=f"kT{t}") for t in range(FT)]
        vp = [persist.tile([P, HL * DV], F16, tag=f"vp{t}", name=f"vp{t}") for t in range(NST)]
        outT = [persist.tile([P, S], F16, tag=f"outT{t}", name=f"outT{t}") for t in range(FT)]

        # ---------------- Stage A: loads + transposes + Q/K, pipelined -----
        with tc.tile_pool(name="xload", bufs=4) as xload, \
             tc.tile_pool(name="tr_psum", bufs=4, space="PSUM") as tr_psum, \
             tc.tile_pool(name="qk_psum", bufs=3, space="PSUM") as qk_psum:

            def qk_chunk(j):
                for which, dst in ((0, qT), (1, kT)):
                    for ft in range(FT):
                        pq = qk_psum.tile([P, 512], FP, tag="mm")
                        for c in range(ET):
                            nc.tensor.matmul(
                                pq[:],
                                (wqkv_sb[c][:, FL * which + P * ft : FL * which + P * (ft + 1)]),
                                (xbT[c][:, 512 * j : 512 * (j + 1)]),
                                start=(c == 0),
                                stop=(c == ET - 1),
                            )
                        nc.vector.tensor_copy(dst[ft][:, 512 * j : 512 * (j + 1)], pq[:])

            for t in range(NST):
                xt = xload.tile([P, E], FP, tag="xt")
                nc.sync.dma_start(xt[:], xb[P * t : P * (t + 1), :])
                for c in range(ET):
                    pt = tr_psum.tile([P, P], FP, tag="tr")
                    nc.tensor.transpose(pt[:], xt[:, P * c : P * (c + 1)], ident[:])
                    nc.vector.tensor_copy(xbT[c][:, P * t : P * (t + 1)], pt[:])
                if t % 4 == 3:
                    qk_chunk(t // 4)

        # ---------------- Stage B: attention ------------------------------
        # head 0 / half 0 carries the woven V projection (one k-tile per
        # group); PSUM there is s(4) + v(2) + o_h0(2) = 8 banks. Afterwards
        # v+o_h0 close and the steady o pool (4 banks) opens.
        with tc.tile_pool(name="s_psum", bufs=2, space="PSUM") as s_psum, \
             tc.tile_pool(name="expst", bufs=8) as expst, \
             tc.tile_pool(name="smalls", bufs=6) as smalls, \
             tc.tile_pool(name="invdram", bufs=6, space="DRAM") as invdram:

            def norm(po, h, half, ht, hb):
                q0 = HQ * half
                for j in range(2):
                    # sums row -> DRAM, reread as [128,4] so the reciprocal
                    # runs 128-wide (free-size cost), then bounce back
                    # through DRAM for the partition broadcast (SBUF
                    # sources can't have step-0 partitions; DRAM can).
                    srow = smalls.tile([1, 512], FP, tag="srow")
                    nc.vector.tensor_copy(srow[:], po[j][D : D + 1, :])
                    sd = invdram.tile([1, 512], FP, tag="sd")
                    nc.sync.dma_start(sd[:], srow[:])
                    s4 = smalls.tile([P, 4], FP, tag="s4")
                    nc.sync.dma_start(
                        s4[:], sd.rearrange("a (p f) -> (a p) f", p=P)
                    )
                    inv4 = smalls.tile([P, 4], FP, tag="inv4")
                    nc.vector.reciprocal(inv4[:], s4[:])
                    invd = invdram.tile([1, 512], FP, tag="invd")
                    nc.sync.dma_start(
                        invd.rearrange("a (p f) -> (a p) f", p=P), inv4[:]
                    )
                    inv64 = smalls.tile([D, 512], FP, tag="inv64")
                    nc.sync.dma_start(
                        inv64[:], invd[0:1, :].to_broadcast((D, 512))
                    )
                    nc.vector.tensor_mul(
                        outT[ht][hb : hb + D, q0 + 512 * j : q0 + 512 * (j + 1)],
                        po[j][0:D, :],
                        inv64[:],
                    )

            def attn_group(h, half, i, ps_pool, o_tiles, es_pool):
                ht, hb = (D * h) // P, (D * h) % P
                q0 = HQ * half
                ps = ps_pool.tile([P, HQ], FP, tag="s")
                for j in range(2):
                    nc.tensor.matmul(
                        ps[:, 512 * j : 512 * (j + 1)],
                        (kT[ht][hb : hb + D, P * i : P * (i + 1)]),
                        (qT[ht][hb : hb + D, q0 + 512 * j : q0 + 512 * (j + 1)]),
                        start=True,
                        stop=True,
                    )
                es = es_pool.tile([P, HQ], F16, tag="e")
                nc.scalar.activation(es[:], ps[:], Exp, scale=SCALE)
                for j in range(2):
                    nc.tensor.matmul(
                        o_tiles[j][:DV, :],
                        vp[i][:, DV * h : DV * (h + 1)],
                        es[:, 512 * j : 512 * (j + 1)],
                        start=(i == 0),
                        stop=(i == NKT - 1),
                    )

            with tc.tile_pool(name="v_psum", bufs=2, space="PSUM") as v_psum, \
                 tc.tile_pool(name="o_h0", bufs=2, space="PSUM") as o_h0:
                po = [o_h0.tile([P, 512], FP, tag="o", name=f"po0_0_{j}") for j in range(2)]
                for i in range(NKT):
                    # V projection for k-tile i, woven ahead of its AV use
                    pv = v_psum.tile([P, 512], FP, tag="v")
                    for c in range(ET):
                        nc.tensor.matmul(
                            pv[:, :FL],
                            (xbT[c][:, P * i : P * (i + 1)]),
                            (wqkv_sb[c][:, 2 * FL : 3 * FL]),
                            start=(c == 0),
                            stop=(c == ET - 1),
                        )
                    v3 = vp[i].rearrange("p (h d) -> p h d", d=DV)
                    nc.vector.tensor_copy(
                        v3[:, :, 0:D], pv[:, :FL].rearrange("p (h d) -> p h d", d=D)
                    )
                    nc.vector.memset(v3[:, :, D : D + 1], 1.0)
                    attn_group(0, 0, i, s_psum, po, expst)
                norm(po, 0, 0, 0, 0)

            with tc.tile_pool(name="o_psum", bufs=4, space="PSUM") as o_psum:
                rest = [(0, 1)] + [(h, half) for h in range(1, HL) for half in range(2)]
                for h, half in rest:
                    ht, hb = (D * h) // P, (D * h) % P
                    po = [
                        o_psum.tile([P, 512], FP, tag="o", name=f"po{h}_{half}_{j}")
                        for j in range(2)
                    ]
                    for i in range(NKT):
                        attn_group(h, half, i, s_psum, po, expst)
                    norm(po, h, half, ht, hb)

        # ---------------- Stage C: partial output projection + bias --------
        with tc.tile_pool(name="f_psum", bufs=2, space="PSUM") as f_psum, \
             tc.tile_pool(name="osb", bufs=4) as osb:
            for t in range(NST):
                pf1 = f_psum.tile([P, 512], FP, tag="f1")
                pf2 = f_psum.tile([P, 512], FP, tag="f2")
                for c in range(FT):
                    nc.tensor.matmul(
                        pf1[:],
                        (outT[c][:, P * t : P * (t + 1)]),
                        (wproj_sb[c][:, 0:512]),
                        start=(c == 0),
                        stop=(c == FT - 1),
                    )
                for c in range(FT):
                    nc.tensor.matmul(
                        pf2[:, :256],
                        (outT[c][:, P * t : P * (t + 1)]),
                        (wproj_sb[c][:, 512:E]),
                        start=(c == 0),
                        stop=(c == FT - 1),
                    )
                ot = osb.tile([P, E], FP, tag="ot")
                nc.vector.tensor_add(ot[:, 0:512], pf1[:], bias_sb[:, 0:512])
                nc.vector.tensor_add(ot[:, 512:E], pf2[:, :256], bias_sb[:, 512:E])
                nc.sync.dma_start(out[P * t : P * (t + 1), :], ot[:])

    _split_waits(nc)
    return nc


_CACHE = {}


def _get_nc():
    if "nc" not in _CACHE:
        _CACHE["nc"] = build()
    return _CACHE["nc"]


def make_in_maps(x, w_qkv, w_proj, b_proj):
    x = np.asarray(x, dtype=np.float32)
    w_qkv = np.asarray(w_qkv, np.float32)
    w_proj = np.asarray(w_proj, np.float32)
    b_proj = np.asarray(b_proj, np.float32)
    bias0 = np.ascontiguousarray(np.broadcast_to(b_proj, (P, E)))
    biasz = np.zeros((P, E), np.float32)
    in_maps = []
    for c in range(NCORES):
        b, half = c // 2, c % 2
        heads = range(HL * half, HL * half + HL)
        rows = (
            [E * 0 + D * h + d for h in heads for d in range(D)]
            + [E * 1 + D * h + d for h in heads for d in range(D)]
            + [E * 2 + D * h + d for h in heads for d in range(D)]
        )
        wqkvT_l = np.ascontiguousarray(w_qkv[rows, :].T).astype(np.float16)
        wprojT_l = np.ascontiguousarray(w_proj[:, rows[: FL]].T).astype(np.float16)
        in_maps.append(
            {
                "identd": np.eye(P, dtype=np.float32),
                "xb": np.ascontiguousarray(x[b]),
                "wqkvT": wqkvT_l,
                "wprojT": wprojT_l,
                "biasb": bias0 if half == 0 else biasz,
            }
        )
    return in_maps


def assemble(results):
    outp = np.empty((B, S, E), np.float32)
    for b in range(B):
        outp[b] = results[2 * b]["out"] + results[2 * b + 1]["out"]
    return outp


def kernel(x, w_qkv, w_proj, b_proj):
    nc = _get_nc()
    in_maps = make_in_maps(x, w_qkv, w_proj, b_proj)
    res = run_bass_kernel_spmd(nc, in_maps, core_ids=list(range(NCORES)))
    return assemble(res.results)
